# revision 16
# baseline (speedup 1.0000x reference)
"""Causal linear attention (elu+1 feature map) for Trainium2, 8-core SPMD.

Sharding: core c = (a, b) with a = c//4 (batch of N=2... batch index a covers
batches {0,1} with 4 cores each) and b = c%4 (head quarter: heads [4b:4b+4)
of 16, i.e. feature columns [256b:256b+256) of the 1024-wide head dim).

Each core:
  P1: projects its batch's inputs through its head-quarter of Wq/Wk/Wv
      (bf16 matmuls, fp32 PSUM accumulation), applying phi(x)=elu(x)+1.
  P2: chunked causal linear attention for its 4 (head, batch) pairs
      (chunk = 128 sequence positions; intra-chunk masked A' matmuls +
      inter-chunk running state S).
  A2A: 8-way AllToAll redistributes attention outputs from
      (head-quarter, full batch-seq) sharding to (l-chunk, all heads).
  P3: output projection with Wo for its 256 output rows.

Host side only slices/transposes/replicates numpy inputs (pure layout) and
reassembles the sharded outputs.
"""

import sys

sys.path.insert(0, "/opt/trn_rl_repo")

import numpy as np

import concourse.bass as bass
import concourse.mybir as mybir
from concourse.tile import TileContext
from concourse.bass_utils import run_bass_kernel_spmd
from concourse.masks import make_upper_triangular

F32 = mybir.dt.float32
BF16 = mybir.dt.bfloat16

L = 1024          # sequence length
NB = 2            # batch
E = 1024          # embed dim
H = 16            # heads
D = 64            # head dim
EPS = 1e-6
N_CORES = 8
FPC = 256         # features per core (4 heads)
C = 128           # chunk size
NCH = L // C      # chunks per (head, batch) pair

LAST_RESULT = None  # set by kernel() for test harnesses


def _split_waits(nc, cap=1):
    """Walrus allows only one sync-wait on pseudo instructions (DMA triggers,
    collective triggers, drains). Move excess waits onto preceding single-wait
    NoOps on the same engine (engine FIFO order keeps semantics identical)."""
    ctr = 0
    for f in nc.m.functions:
        for blk in f.blocks:
            insts = list(blk.instructions)
            new = []
            changed = False
            for ins in insts:
                si = ins.sync_info
                waits = list(si.on_wait) if (si and si.on_wait) else []
                if len(waits) > cap:
                    excess, keep = waits[:-cap], waits[-cap:]
                    for w in excess:
                        new.append(mybir.InstNoOp(
                            name=f"wsplit-{ctr}",
                            sync_info=mybir.SyncInfo(on_wait=[w], on_update=[]),
                            engine=ins.engine,
                            bass_nofuse=True,
                        ))
                        ctr += 1
                    ins.sync_info = mybir.SyncInfo(on_wait=keep, on_update=si.on_update)
                    changed = True
                new.append(ins)
            if changed:
                blk.instructions = new
    return ctr


def _build(with_bias):
    nc = bass.Bass(num_devices=N_CORES)

    # Per-core external inputs (host feeds core-specific slices).
    xqT = nc.declare_dram_parameter("xqT", [E, L], F32, isOutput=False)
    xkT = nc.declare_dram_parameter("xkT", [E, L], F32, isOutput=False)
    xvT = nc.declare_dram_parameter("xvT", [E, L], F32, isOutput=False)
    wqT = nc.declare_dram_parameter("wqT", [E, FPC], F32, isOutput=False)
    wkT = nc.declare_dram_parameter("wkT", [E, FPC], F32, isOutput=False)
    wvT = nc.declare_dram_parameter("wvT", [E, FPC], F32, isOutput=False)
    woT = nc.declare_dram_parameter("woT", [E, E], F32, isOutput=False)
    if with_bias:
        bq_d = nc.declare_dram_parameter("bq", [FPC, 1], F32, isOutput=False)
        bk_d = nc.declare_dram_parameter("bk", [FPC, 1], F32, isOutput=False)
        bv_d = nc.declare_dram_parameter("bv", [FPC, 1], F32, isOutput=False)
        bo_d = nc.declare_dram_parameter("bo", [1, E], F32, isOutput=False)
    out_d = nc.declare_dram_parameter("out", [NB, C, E], F32, isOutput=True)
    # token passthrough: lets a timing harness chain executions serially
    tok_i = nc.declare_dram_parameter("tok", [1, 1], F32, isOutput=False)
    tok_o = nc.declare_dram_parameter("tok_out", [1, 1], F32, isOutput=True)

    with TileContext(nc) as tc:
        with (
            tc.tile_pool(name="const", bufs=1) as constp,
            tc.tile_pool(name="xT", bufs=1) as xTp,
            tc.tile_pool(name="wT", bufs=1) as wTp,
            tc.tile_pool(name="woTp", bufs=1) as woTp,
            tc.tile_pool(name="proj", bufs=1) as projp,
            tc.tile_pool(name="tmp", bufs=3) as tmpp,
            tc.tile_pool(name="p2", bufs=6) as p2p
            ,tc.tile_pool(name="atT", bufs=1) as atTp,
            tc.tile_pool(name="attn", bufs=1) as attnp,
            tc.tile_pool(name="outp", bufs=1) as outp,
            tc.tile_pool(name="dram", bufs=1, space="DRAM") as dram,
            tc.tile_pool(name="ps_pj", bufs=2, space="PSUM") as ps_pj,
            tc.tile_pool(name="ps_pn", bufs=2, space="PSUM") as ps_pn,
            tc.tile_pool(name="ps_ao", bufs=3, space="PSUM") as ps_ao,
            tc.tile_pool(name="ps_sh", bufs=1, space="PSUM") as ps_sh,
        ):
            # ---------------- constants ----------------
            maskU = constp.tile([C, C], F32)          # mask[s,t] = 1 if s<=t else 0
            make_upper_triangular(nc, maskU[:], val=1.0, diag=True)
            ones_row = constp.tile([1, D], F32)       # lhsT for denom broadcast mm
            nc.vector.memset(ones_row[:], 1.0)

            if with_bias:
                bqs = [constp.tile([C, 1], F32, tag=f"bq{m}", name=f"bqs{m}") for m in range(2)]
                bks = [constp.tile([C, 1], F32, tag=f"bk{m}", name=f"bks{m}") for m in range(2)]
                for m in range(2):
                    nc.sync.dma_start(out=bqs[m][:], in_=bq_d[m * C:(m + 1) * C])
                    nc.sync.dma_start(out=bks[m][:], in_=bk_d[m * C:(m + 1) * C])
                bo_row = constp.tile([1, E], F32)
                nc.sync.dma_start(out=bo_row[:], in_=bo_d[:])
                bkr = constp.tile([1, FPC], F32)
                bvr = constp.tile([1, FPC], F32)
                nc.sync.dma_start(out=bkr[:], in_=bk_d[:].rearrange("f one -> one f"))
                nc.sync.dma_start(out=bvr[:], in_=bv_d[:].rearrange("f one -> one f"))
                ones_row1 = constp.tile([1, C], F32)
                nc.vector.memset(ones_row1[:], 1.0)
                # broadcast-across-partitions tiles via K=1 matmuls
                bo_bc = constp.tile([C, E], F32)
                for nbk in range(2):
                    bo_ps = ps_pj.tile([C, 512], F32, tag="pj")
                    nc.tensor.matmul(bo_ps[:, 0:512], ones_row1[:, 0:C],
                                     bo_row[:, nbk * 512:(nbk + 1) * 512],
                                     start=True, stop=True)
                    nc.vector.tensor_copy(bo_bc[:, nbk * 512:(nbk + 1) * 512], bo_ps[:])
                bk_bc = constp.tile([C, FPC], F32)
                bv_bc = constp.tile([C, FPC], F32)
                for row, bc in ((bkr, bk_bc), (bvr, bv_bc)):
                    bps = ps_pn.tile([C, FPC], F32, tag="pn")
                    nc.tensor.matmul(bps[:], ones_row1[:, 0:C], row[:],
                                     start=True, stop=True)
                    nc.vector.tensor_copy(bc[:], bps[:])

            # ---------------- input loads (cast fp32 -> bf16) ----------------
            # one big cast-DMA per tensor; k-tile k lives at [:, k, :]
            def big_load(pool, src, kd, fd, nm, halves=1):
                t = pool.tile([C, kd, fd], BF16, tag=nm, name=nm)
                src_r = src[:].rearrange("(k p) f -> p k f", p=C)
                step = kd // halves
                for h in range(halves):
                    ks = slice(h * step, (h + 1) * step)
                    nc.gpsimd.dma_start(out=t[:, ks, :], in_=src_r[:, ks, :])
                return t
            wv_all = big_load(wTp, wvT, 8, FPC, "wv_all")
            wk_all = big_load(wTp, wkT, 8, FPC, "wk_all")
            wq_all = big_load(wTp, wqT, 8, FPC, "wq_all")
            xv_all = big_load(xTp, xvT, 8, L, "xv_all", halves=2)
            xk_all = big_load(xTp, xkT, 8, L, "xk_all", halves=2)
            xq_all = big_load(xTp, xqT, 8, L, "xq_all", halves=2)
            xv_sb = [xv_all[:, k, :] for k in range(8)]
            xk_sb = [xk_all[:, k, :] for k in range(8)]
            xq_sb = [xq_all[:, k, :] for k in range(8)]
            wv_sb = [wv_all[:, k, :] for k in range(8)]
            wk_sb = [wk_all[:, k, :] for k in range(8)]
            wq_sb = [wq_all[:, k, :] for k in range(8)]

            # ---------------- P1a: natural-layout k_nat / v_nat ----------------
            # v_nat tiles (128 l, 4 pairs x 65) with ones column appended per pair
            # k_nat tiles (128 l, 256 f)
            vnat, knat = [], []
            for lt in range(8):
                vt = projp.tile([C, 4 * 65], BF16, tag=f"vn{lt}")
                nc.vector.memset(vt[:], 1.0)   # ones columns (and init)
                ps = ps_pn.tile([C, FPC], F32, tag="pn")
                for k in range(8):
                    nc.tensor.matmul(ps[:], xv_sb[k][:, lt * C:(lt + 1) * C],
                                     wv_sb[k][:], start=(k == 0), stop=(k == 7))
                vna = vt[:].rearrange("p (f c) -> p f c", f=4)
                if with_bias:
                    nc.vector.tensor_add(vna[:, :, 0:64],
                                         ps[:].rearrange("p (f c) -> p f c", f=4),
                                         bv_bc[:].rearrange("p (f c) -> p f c", f=4))
                else:
                    nc.scalar.copy(vna[:, :, 0:64],
                                   ps[:].rearrange("p (f c) -> p f c", f=4))
                vnat.append(vt)

                kt = projp.tile([C, FPC], BF16, tag=f"kn{lt}")
                psk = ps_pn.tile([C, FPC], F32, tag="pn")
                for k in range(8):
                    nc.tensor.matmul(psk[:], xk_sb[k][:, lt * C:(lt + 1) * C],
                                     wk_sb[k][:], start=(k == 0), stop=(k == 7))
                if with_bias:
                    psb = tmpp.tile([C, FPC], F32, tag="psb")
                    nc.vector.tensor_add(psb[:], psk[:], bk_bc[:])
                    psk = psb
                # phi(x) = max(exp(min(x,0)), x+1), elementwise
                tmin = tmpp.tile([C, FPC], F32, tag="tmin")
                nc.vector.tensor_scalar(tmin[:], psk[:], 0.0, None,
                                        op0=mybir.AluOpType.min)
                texp = tmpp.tile([C, FPC], F32, tag="texp")
                nc.scalar.activation(texp[:], tmin[:],
                                     mybir.ActivationFunctionType.Exp)
                tu = tmpp.tile([C, FPC], F32, tag="tu")
                nc.scalar.add(tu[:], psk[:], 1.0)
                nc.vector.tensor_max(kt[:], texp[:], tu[:])
                knat.append(kt)

            # ---------------- P1b: feature-major q' / k' ----------------
            # per-pair tiles (64, L) bf16, base partition 0
            qp_t = [projp.tile([D, L], BF16, tag=f"qp{p}", name=f"qp{p}") for p in range(4)]
            kp_t = [projp.tile([D, L], BF16, tag=f"kp{p}", name=f"kp{p}") for p in range(4)]
            for which, wsb, xsb, dst in (("q", wq_sb, xq_sb, qp_t),
                                         ("k", wk_sb, xk_sb, kp_t)):
                for m in range(2):
                    for nbk in range(2):
                        ps = ps_pj.tile([C, 512], F32, tag="pj")
                        for k in range(8):
                            nc.tensor.matmul(
                                ps[:], wsb[k][:, m * C:(m + 1) * C],
                                xsb[k][:, nbk * 512:(nbk + 1) * 512],
                                start=(k == 0), stop=(k == 7))
                        cs = slice(nbk * 512, (nbk + 1) * 512)
                        tmin = tmpp.tile([C, 512], F32, tag="tmin")
                        if with_bias:
                            bias = (bqs if which == "q" else bks)[m][:]
                            nc.vector.tensor_scalar(
                                tmin[:], ps[:], bias, 0.0,
                                op0=mybir.AluOpType.add, op1=mybir.AluOpType.min)
                        else:
                            nc.vector.tensor_scalar(
                                tmin[:], ps[:], 0.0, None, op0=mybir.AluOpType.min)
                        texp = tmpp.tile([C, 512], F32, tag="texp")
                        nc.scalar.activation(texp[:], tmin[:],
                                             mybir.ActivationFunctionType.Exp)
                        tu = tmpp.tile([C, 512], F32, tag="tu")
                        if with_bias:
                            bias = (bqs if which == "q" else bks)[m][:]
                            nc.vector.tensor_scalar(
                                tu[:], ps[:], bias, 1.0,
                                op0=mybir.AluOpType.add, op1=mybir.AluOpType.add)
                        else:
                            nc.scalar.add(tu[:], ps[:], 1.0)
                        for half in range(2):
                            pr = slice(half * D, (half + 1) * D)
                            nc.vector.tensor_max(dst[2 * m + half][:, cs],
                                                 texp[pr, 0:512], tu[pr, 0:512])

            # ---------------- P2: chunked causal linear attention ----------------
            # attnA: pairs 0,1 (feature-major (128, L)); attnB: pairs 2,3
            attnA = attnp.tile([C, L], BF16, tag="attnA")
            attnB = attnp.tile([C, L], BF16, tag="attnB")
            for p in range(4):
                att = attnA if p < 2 else attnB
                arow = slice((p % 2) * D, (p % 2) * D + D)
                vcols = slice(p * 65, p * 65 + 65)
                kcols = slice(p * D, (p + 1) * D)
                shat = ps_sh.tile([D, 65], F32, tag="sh")
                ssb_prev = None
                for cI in range(NCH):
                    cs = slice(cI * C, (cI + 1) * C)
                    # A'[s,t] = sum_d K'[d,s] Q'[d,t]  (then causal mask s<=t)
                    aps = ps_ao.tile([C, C], F32, tag="ao")
                    nc.tensor.matmul(aps[:], kp_t[p][:, cs], qp_t[p][:, cs],
                                     start=True, stop=True)
                    asb = p2p.tile([C, C], BF16, tag="asb")
                    nc.vector.tensor_mul(asb[:], aps[:], maskU[:])
                    # O' (65, 128) = V^.T @ A' (+ S^.T @ Q' for c>0)
                    ops = ps_ao.tile([65, C], F32, tag="ao")
                    nc.tensor.matmul(ops[:], vnat[cI][:, vcols], asb[:],
                                     start=True, stop=(cI == 0))
                    if cI > 0:
                        nc.tensor.matmul(ops[:], ssb_prev[:], qp_t[p][:, cs],
                                         start=False, stop=True)
                    # running state S^ += K_c^T @ V^_c  (not needed after last)
                    if cI < NCH - 1:
                        nc.tensor.matmul(shat[:], knat[cI][:, kcols],
                                         vnat[cI][:, vcols],
                                         start=(cI == 0), stop=(cI == NCH - 2))
                        ssb = p2p.tile([D, 65], BF16, tag="ssb")
                        nc.scalar.copy(ssb[:], shat[:])
                        ssb_prev = ssb
                    # divide: attn' = O'[0:64] * (1/(denom+eps)) broadcast
                    rsb = p2p.tile([1, C], F32, tag="rsb")
                    nc.vector.tensor_scalar(rsb[:], ops[64:65, :], EPS, None,
                                            op0=mybir.AluOpType.add)
                    rcp = p2p.tile([1, C], F32, tag="rcp")
                    nc.vector.reciprocal(rcp[:], rsb[:])
                    rbc_ps = ps_ao.tile([D, C], F32, tag="ao")
                    nc.tensor.matmul(rbc_ps[:], ones_row[:], rcp[:],
                                     start=True, stop=True)
                    rbc = p2p.tile([D, C], F32, tag="rbc")
                    nc.scalar.copy(rbc[:], rbc_ps[:])
                    nc.vector.tensor_mul(att[arow, cs], ops[0:64, :], rbc[:])

            # ---------------- A2A: redistribute attention outputs ----------------
            # shard j of the input buffer: (128 feats, l-chunk j), one DMA each
            a2a_outs = []
            for att in (attnA, attnB):
                a2a_in = dram.tile([N_CORES * C, C], BF16,
                                   tag=f"a2ain{len(a2a_outs)}")
                a2a_out = dram.tile([N_CORES * C, C], BF16,
                                    tag=f"a2aout{len(a2a_outs)}")
                nc.sync.dma_start(
                    out=a2a_in[:].rearrange("(j f) l -> f j l", j=N_CORES),
                    in_=att[:].rearrange("f (j l) -> f j l", j=N_CORES),
                )
                nc.gpsimd.collective_compute(
                    "AllToAll",
                    mybir.AluOpType.bypass,
                    replica_groups=[list(range(N_CORES))],
                    ins=[a2a_in.opt()],
                    outs=[a2a_out.opt()],
                )
                a2a_outs.append(a2a_out)

            # ---------------- woT load (single big cast-DMA) ----------------
            wo_all = big_load(woTp, woT, 8, E, "wo_all")
            wo_sb = [wo_all[:, k, :] for k in range(8)]

            # ---------------- P3: output projection ----------------
            for n in range(NB):
                # one DMA per half: 4 contiguous shards (cores n*4..n*4+3)
                ath = []
                for half in range(2):
                    t = atTp.tile([C, 4, C], BF16, tag=f"ath{n}_{half}",
                                  name=f"ath{n}_{half}")
                    nc.sync.dma_start(
                        out=t[:],
                        in_=a2a_outs[half][n * 4 * C:(n + 1) * 4 * C, :]
                            .rearrange("(s p) l -> p s l", p=C))
                    ath.append(t)
                at_tiles = [ath[kk % 2][:, kk // 2, :] for kk in range(8)]
                kk_order = [0, 2, 4, 6, 1, 3, 5, 7]
                osb = outp.tile([C, E], F32, tag=f"osb{n}")
                for nbk in range(2):
                    ps = ps_pj.tile([C, 512], F32, tag="pj")
                    for i, kk in enumerate(kk_order):
                        nc.tensor.matmul(ps[:], at_tiles[kk][:],
                                         wo_sb[kk][:, nbk * 512:(nbk + 1) * 512],
                                         start=(i == 0), stop=(i == 7))
                    cs = slice(nbk * 512, (nbk + 1) * 512)
                    if with_bias:
                        nc.vector.tensor_add(osb[:, cs], ps[:], bo_bc[:, cs])
                    else:
                        nc.scalar.copy(osb[:, cs], ps[:])
                nc.sync.dma_start(out=out_d[n], in_=osb[:])
            nc.sync.dma_start(out=tok_o[:], in_=tok_i[:])

    _split_waits(nc)
    return nc


def _run_pjrt_timed(nc, in_maps, time_iters=0):
    """Replicates bass2jax.run_bass_via_pjrt's multi-core path, but keeps
    inputs device-resident and (optionally) times repeated executions.
    Returns (results, best_exec_seconds_or_None)."""
    import time as _time
    import jax
    from jax.sharding import Mesh, PartitionSpec, NamedSharding
    from jax.experimental.shard_map import shard_map
    from concourse import bass2jax, mybir as mb

    bass2jax.install_neuronx_cc_hook()
    n_cores = len(in_maps)
    partition_name = nc.partition_id_tensor.name if nc.partition_id_tensor else None

    in_names, out_names, out_avals, zero_outs = [], [], [], []
    for alloc in nc.m.functions[0].allocations:
        if not isinstance(alloc, mb.MemoryLocationSet):
            continue
        name = alloc.memorylocations[0].name
        if alloc.kind == "ExternalInput":
            if name != partition_name:
                in_names.append(name)
        elif alloc.kind == "ExternalOutput":
            out_names.append(name)
            shape = tuple(alloc.tensor_shape)
            dtype = mb.dt.np(alloc.dtype)
            out_avals.append(jax.core.ShapedArray(shape, dtype))
            zero_outs.append(np.zeros(shape, dtype))
    n_params = len(in_names)
    in_names.extend(out_names)
    if partition_name is not None:
        in_names.append(partition_name)

    chain = int(__import__("os").environ.get("TRN_KERNEL_CHAIN", "1"))
    tok_in_idx = in_names.index("tok") if "tok" in in_names else None
    tok_out_idx = out_names.index("tok_out") if "tok_out" in out_names else None

    def _body(*args):
        operands = list(args)
        pid = bass2jax.partition_id_tensor() if partition_name is not None else None
        outs = None
        for _ in range(chain):
            ops = list(operands)
            if outs is not None and tok_in_idx is not None:
                ops[tok_in_idx] = outs[tok_out_idx]  # serialize iterations
            if pid is not None:
                ops.append(pid)
            outs = bass2jax._bass_exec_p.bind(
                *ops,
                out_avals=tuple(out_avals),
                in_names=tuple(in_names),
                out_names=tuple(out_names),
                lowering_input_output_aliases=(),
                sim_require_finite=True,
                sim_require_nnan=True,
                nc=nc,
            )
        return tuple(outs)

    devices = jax.devices()[:n_cores]
    mesh = Mesh(np.asarray(devices), ("core",))
    in_specs = (PartitionSpec("core"),) * (n_params + len(out_names))
    out_specs = (PartitionSpec("core"),) * len(out_names)
    sharded = jax.jit(
        shard_map(_body, mesh=mesh, in_specs=in_specs, out_specs=out_specs,
                  check_rep=False),
        keep_unused=True,
    )
    per_core = [[np.asarray(m[name]) for name in in_names[:n_params]]
                for m in in_maps]
    concat_in = [np.concatenate([per_core[c][i] for c in range(n_cores)], axis=0)
                 for i in range(n_params)]
    concat_zeros = [np.zeros((n_cores * z.shape[0], *z.shape[1:]), z.dtype)
                    for z in zero_outs]
    shd = NamedSharding(mesh, PartitionSpec("core"))
    dev_in = [jax.device_put(a, shd) for a in concat_in + concat_zeros]

    out_arrs = sharded(*dev_in)
    jax.block_until_ready(out_arrs)
    best = None
    for _ in range(time_iters):
        t0 = _time.perf_counter()
        out_arrs2 = sharded(*dev_in)
        jax.block_until_ready(out_arrs2)
        dt = _time.perf_counter() - t0
        best = dt if best is None or dt < best else best
    results = [
        {name: np.asarray(out_arrs[i]).reshape(n_cores, *out_avals[i].shape)[c]
         for i, name in enumerate(out_names)}
        for c in range(n_cores)
    ]
    return results, best


def kernel(**inputs):
    global LAST_RESULT
    import os

    query = np.asarray(inputs["query"], np.float32)
    key = np.asarray(inputs["key"], np.float32)
    value = np.asarray(inputs["value"], np.float32)
    Wq = np.asarray(inputs["Wq"], np.float32)
    Wk = np.asarray(inputs["Wk"], np.float32)
    Wv = np.asarray(inputs["Wv"], np.float32)
    Wo = np.asarray(inputs["Wo"], np.float32)
    bq = np.asarray(inputs["bq"], np.float32)
    bk = np.asarray(inputs["bk"], np.float32)
    bv = np.asarray(inputs["bv"], np.float32)
    bo = np.asarray(inputs["bo"], np.float32)

    with_bias = any(np.any(b) for b in (bq, bk, bv, bo))
    nc = _build(with_bias)

    woT_full = np.ascontiguousarray(Wo.T)
    in_maps = []
    for c in range(N_CORES):
        a, b = c // 4, c % 4
        F = slice(FPC * b, FPC * (b + 1))
        m = {
            "xqT": np.ascontiguousarray(query[:, a, :].T),
            "xkT": np.ascontiguousarray(key[:, a, :].T),
            "xvT": np.ascontiguousarray(value[:, a, :].T),
            "wqT": np.ascontiguousarray(Wq[F, :].T),
            "wkT": np.ascontiguousarray(Wk[F, :].T),
            "wvT": np.ascontiguousarray(Wv[F, :].T),
            "woT": woT_full,
            "tok": np.zeros((1, 1), np.float32),
        }
        if with_bias:
            m["bq"] = np.ascontiguousarray(bq[F].reshape(FPC, 1))
            m["bk"] = np.ascontiguousarray(bk[F].reshape(FPC, 1))
            m["bv"] = np.ascontiguousarray(bv[F].reshape(FPC, 1))
            m["bo"] = np.ascontiguousarray(bo.reshape(1, E))
        in_maps.append(m)

    time_iters = int(os.environ.get("TRN_KERNEL_TIME_ITERS", "0"))
    results, best = _run_pjrt_timed(nc, in_maps, time_iters=time_iters)
    LAST_RESULT = {"results": results, "best_exec_s": best}

    out = np.empty((L, NB, E), np.float32)
    for c in range(N_CORES):
        o = results[c]["out"]  # (NB, C, E): my l-chunk rows for both batches
        for n in range(NB):
            out[c * C:(c + 1) * C, n, :] = o[n]
    return out


# revision 26
# speedup vs baseline: 687.2923x; 687.2923x over previous
"""Causal linear attention (elu+1 feature map) for Trainium2, 8-core SPMD.

Sharding: core c = (a, b) with a = c//4 (batch of N=2... batch index a covers
batches {0,1} with 4 cores each) and b = c%4 (head quarter: heads [4b:4b+4)
of 16, i.e. feature columns [256b:256b+256) of the 1024-wide head dim).

Each core:
  P1: projects its batch's inputs through its head-quarter of Wq/Wk/Wv
      (bf16 matmuls, fp32 PSUM accumulation), applying phi(x)=elu(x)+1.
  P2: chunked causal linear attention for its 4 (head, batch) pairs
      (chunk = 128 sequence positions; intra-chunk masked A' matmuls +
      inter-chunk running state S).
  A2A: 8-way AllToAll redistributes attention outputs from
      (head-quarter, full batch-seq) sharding to (l-chunk, all heads).
  P3: output projection with Wo for its 256 output rows.

Host side only slices/transposes/replicates numpy inputs (pure layout) and
reassembles the sharded outputs.
"""

import sys

sys.path.insert(0, "/opt/trn_rl_repo")

import numpy as np

import concourse.bass as bass
import concourse.mybir as mybir
from concourse.tile import TileContext
from concourse.bass_utils import run_bass_kernel_spmd
from concourse.masks import make_upper_triangular

F32 = mybir.dt.float32
BF16 = mybir.dt.bfloat16

L = 1024          # sequence length
NB = 2            # batch
E = 1024          # embed dim
H = 16            # heads
D = 64            # head dim
EPS = 1e-6
N_CORES = 8
FPC = 256         # features per core (4 heads)
C = 128           # chunk size
NCH = L // C      # chunks per (head, batch) pair

LAST_RESULT = None  # set by kernel() for test harnesses


def _split_waits(nc, cap=1):
    """Walrus allows only one sync-wait on pseudo instructions (DMA triggers,
    collective triggers, drains). Move excess waits onto preceding single-wait
    NoOps on the same engine (engine FIFO order keeps semantics identical)."""
    ctr = 0
    for f in nc.m.functions:
        for blk in f.blocks:
            insts = list(blk.instructions)
            new = []
            changed = False
            for ins in insts:
                si = ins.sync_info
                waits = list(si.on_wait) if (si and si.on_wait) else []
                if len(waits) > cap:
                    excess, keep = waits[:-cap], waits[-cap:]
                    for w in excess:
                        new.append(mybir.InstNoOp(
                            name=f"wsplit-{ctr}",
                            sync_info=mybir.SyncInfo(on_wait=[w], on_update=[]),
                            engine=ins.engine,
                            bass_nofuse=True,
                        ))
                        ctr += 1
                    ins.sync_info = mybir.SyncInfo(on_wait=keep, on_update=si.on_update)
                    changed = True
                new.append(ins)
            if changed:
                blk.instructions = new
    return ctr


def _build(with_bias):
    nc = bass.Bass(num_devices=N_CORES)

    # Per-core external inputs (host feeds core-specific slices).
    xqT = nc.declare_dram_parameter("xqT", [E, L], F32, isOutput=False)
    xkT = nc.declare_dram_parameter("xkT", [E, L], F32, isOutput=False)
    xvT = nc.declare_dram_parameter("xvT", [E, L], F32, isOutput=False)
    wqT = nc.declare_dram_parameter("wqT", [E, FPC], F32, isOutput=False)
    wkT = nc.declare_dram_parameter("wkT", [E, FPC], F32, isOutput=False)
    wvT = nc.declare_dram_parameter("wvT", [E, FPC], F32, isOutput=False)
    woT = nc.declare_dram_parameter("woT", [E, E], F32, isOutput=False)
    if with_bias:
        bq_d = nc.declare_dram_parameter("bq", [FPC, 1], F32, isOutput=False)
        bk_d = nc.declare_dram_parameter("bk", [FPC, 1], F32, isOutput=False)
        bv_d = nc.declare_dram_parameter("bv", [FPC, 1], F32, isOutput=False)
        bo_d = nc.declare_dram_parameter("bo", [1, E], F32, isOutput=False)
    out_d = nc.declare_dram_parameter("out", [NB, C, E], F32, isOutput=True)
    # token passthrough: lets a timing harness chain executions serially
    tok_i = nc.declare_dram_parameter("tok", [1, 1], F32, isOutput=False)
    tok_o = nc.declare_dram_parameter("tok_out", [1, 1], F32, isOutput=True)

    with TileContext(nc) as tc:
        with (
            tc.tile_pool(name="const", bufs=1) as constp,
            tc.tile_pool(name="xT", bufs=1) as xTp,
            tc.tile_pool(name="wT", bufs=1) as wTp,
            tc.tile_pool(name="woTp", bufs=1) as woTp,
            tc.tile_pool(name="proj", bufs=1) as projp,
            tc.tile_pool(name="tmp", bufs=3) as tmpp,
            tc.tile_pool(name="p2", bufs=6) as p2p
            ,tc.tile_pool(name="atT", bufs=1) as atTp,
            tc.tile_pool(name="attn", bufs=1) as attnp,
            tc.tile_pool(name="outp", bufs=1) as outp,
            tc.tile_pool(name="dram", bufs=1, space="DRAM") as dram,
            tc.tile_pool(name="ps_pj", bufs=2, space="PSUM") as ps_pj,
            tc.tile_pool(name="ps_ao", bufs=5, space="PSUM") as ps_ao,
            tc.tile_pool(name="ps_sh", bufs=1, space="PSUM") as ps_sh,
        ):
            # ---------------- constants ----------------
            maskU = constp.tile([C, C], F32)          # mask[s,t] = 1 if s<=t else 0
            make_upper_triangular(nc, maskU[:], val=1.0, diag=True)
            mask2 = constp.tile([C, 4 * C], F32)      # 4x [maskU]
            for _h in range(4):
                make_upper_triangular(nc, mask2[:, _h * C:(_h + 1) * C],
                                      val=1.0, diag=True)
            ones_row = constp.tile([1, D], F32)       # lhsT for denom broadcast mm
            nc.vector.memset(ones_row[:], 1.0)

            if with_bias:
                bqs = [constp.tile([C, 1], F32, tag=f"bq{m}", name=f"bqs{m}") for m in range(2)]
                bks = [constp.tile([C, 1], F32, tag=f"bk{m}", name=f"bks{m}") for m in range(2)]
                bq1 = [constp.tile([C, 1], F32, tag=f"bq1{m}", name=f"bq1{m}") for m in range(2)]
                bk1 = [constp.tile([C, 1], F32, tag=f"bk1{m}", name=f"bk1{m}") for m in range(2)]
                for m in range(2):
                    nc.sync.dma_start(out=bqs[m][:], in_=bq_d[m * C:(m + 1) * C])
                    nc.sync.dma_start(out=bks[m][:], in_=bk_d[m * C:(m + 1) * C])
                    nc.vector.tensor_scalar(bq1[m][:], bqs[m][:], 1.0, None,
                                            op0=mybir.AluOpType.add)
                    nc.vector.tensor_scalar(bk1[m][:], bks[m][:], 1.0, None,
                                            op0=mybir.AluOpType.add)
                bo_row = constp.tile([1, E], F32)
                nc.sync.dma_start(out=bo_row[:], in_=bo_d[:])
                bkr = constp.tile([1, FPC], F32)
                bvr = constp.tile([1, FPC], F32)
                nc.sync.dma_start(out=bkr[:], in_=bk_d[:].rearrange("f one -> one f"))
                nc.sync.dma_start(out=bvr[:], in_=bv_d[:].rearrange("f one -> one f"))
                ones_row1 = constp.tile([1, C], F32)
                nc.vector.memset(ones_row1[:], 1.0)
                # broadcast-across-partitions tiles via K=1 matmuls
                bo_bc = constp.tile([C, E], F32)
                for nbk in range(2):
                    bo_ps = ps_pj.tile([C, 512], F32, tag="pj")
                    nc.tensor.matmul(bo_ps[:, 0:512], ones_row1[:, 0:C],
                                     bo_row[:, nbk * 512:(nbk + 1) * 512],
                                     start=True, stop=True)
                    nc.vector.tensor_copy(bo_bc[:, nbk * 512:(nbk + 1) * 512], bo_ps[:])
                bk_bc = constp.tile([C, FPC], F32)
                bv_bc = constp.tile([C, FPC], F32)
                for row, bc in ((bkr, bk_bc), (bvr, bv_bc)):
                    bps = ps_pj.tile([C, FPC], F32, tag="pj")
                    nc.tensor.matmul(bps[:], ones_row1[:, 0:C], row[:],
                                     start=True, stop=True)
                    nc.vector.tensor_copy(bc[:], bps[:])

            # ---------------- input loads (cast fp32 -> bf16) ----------------
            # one big cast-DMA per tensor; k-tile k lives at [:, k, :]
            def big_load(pool, src, kd, fd, nm, halves=1):
                t = pool.tile([C, kd, fd], BF16, tag=nm, name=nm)
                src_r = src[:].rearrange("(k p) f -> p k f", p=C)
                step = kd // halves
                for h in range(halves):
                    ks = slice(h * step, (h + 1) * step)
                    nc.gpsimd.dma_start(out=t[:, ks, :], in_=src_r[:, ks, :])
                return t
            wv_all = big_load(wTp, wvT, 8, FPC, "wv_all")
            wk_all = big_load(wTp, wkT, 8, FPC, "wk_all")
            xv_all = big_load(xTp, xvT, 8, L, "xv_all", halves=4)
            xk_all = big_load(xTp, xkT, 8, L, "xk_all", halves=4)
            wq_all = big_load(wTp, wqT, 8, FPC, "wq_all")
            xq_all = big_load(xTp, xqT, 8, L, "xq_all", halves=2)
            xv_sb = [xv_all[:, k, :] for k in range(8)]
            xk_sb = [xk_all[:, k, :] for k in range(8)]
            xq_sb = [xq_all[:, k, :] for k in range(8)]
            wv_sb = [wv_all[:, k, :] for k in range(8)]
            wk_sb = [wk_all[:, k, :] for k in range(8)]
            wq_sb = [wq_all[:, k, :] for k in range(8)]

            # ---------------- P1a: natural-layout k_nat / v_nat ----------------
            # v_nat tiles (128 l, 4 pairs x 65) with ones column appended per pair
            # k_nat tiles (128 l, 256 f)
            vnat, knat = [], []
            for lt in range(8):
                vt = projp.tile([C, 4 * 65], BF16, tag=f"vn{lt}")
                nc.vector.memset(vt[:], 1.0)   # ones columns (and init)
                ps = ps_pj.tile([C, FPC], F32, tag="pj")
                for k in range(8):
                    nc.tensor.matmul(ps[:], xv_sb[k][:, lt * C:(lt + 1) * C],
                                     wv_sb[k][:], start=(k == 0), stop=(k == 7))
                vna = vt[:].rearrange("p (f c) -> p f c", f=4)
                if with_bias:
                    nc.vector.tensor_add(vna[:, :, 0:64],
                                         ps[:].rearrange("p (f c) -> p f c", f=4),
                                         bv_bc[:].rearrange("p (f c) -> p f c", f=4))
                else:
                    nc.scalar.copy(vna[:, :, 0:64],
                                   ps[:].rearrange("p (f c) -> p f c", f=4))
                vnat.append(vt)

                kt = projp.tile([C, FPC], BF16, tag=f"kn{lt}")
                psk = ps_pj.tile([C, FPC], F32, tag="pj")
                for k in range(8):
                    nc.tensor.matmul(psk[:], xk_sb[k][:, lt * C:(lt + 1) * C],
                                     wk_sb[k][:], start=(k == 0), stop=(k == 7))
                if with_bias:
                    psb = tmpp.tile([C, FPC], F32, tag="psb")
                    nc.vector.tensor_add(psb[:], psk[:], bk_bc[:])
                    psk = psb
                # phi(x) = max(exp(min(x,0)), x+1); single PSUM read:
                # u = x+1 (ACT), relu(-x) = relu(1-u), exp(min(x,0)) = exp(-relu(-x))
                tu = tmpp.tile([C, FPC], F32, tag="tu")
                nc.vector.tensor_scalar(tu[:], psk[:], 1.0, None,
                                        op0=mybir.AluOpType.add)
                tmin = tmpp.tile([C, FPC], F32, tag="tmin")
                nc.scalar.activation(tmin[:], tu[:],
                                     mybir.ActivationFunctionType.Relu,
                                     scale=-1.0, bias=1.0)
                texp = tmpp.tile([C, FPC], F32, tag="texp")
                nc.scalar.activation(texp[:], tmin[:],
                                     mybir.ActivationFunctionType.Exp,
                                     scale=-1.0)
                nc.vector.tensor_max(kt[:], texp[:], tu[:])
                knat.append(kt)

            # ---------------- P1b: feature-major q' / k' ----------------
            # per-pair tiles (64, L) bf16, base partition 0
            qp_t = [projp.tile([D, L], BF16, tag=f"qp{p}", name=f"qp{p}") for p in range(4)]
            kp_t = [projp.tile([D, L], BF16, tag=f"kp{p}", name=f"kp{p}") for p in range(4)]
            for which, wsb, xsb, dst in (("q", wq_sb, xq_sb, qp_t),
                                         ("k", wk_sb, xk_sb, kp_t)):
                for m in range(2):
                    for nbk in range(2):
                        ps = ps_pj.tile([C, 512], F32, tag="pj")
                        for k in range(8):
                            nc.tensor.matmul(
                                ps[:], wsb[k][:, m * C:(m + 1) * C],
                                xsb[k][:, nbk * 512:(nbk + 1) * 512],
                                start=(k == 0), stop=(k == 7))
                        cs = slice(nbk * 512, (nbk + 1) * 512)
                        tu = tmpp.tile([C, 512], F32, tag="tu")
                        if with_bias:
                            bias1 = (bq1 if which == "q" else bk1)[m][:]
                            nc.vector.tensor_scalar(tu[:], ps[:], bias1, None,
                                                    op0=mybir.AluOpType.add)
                        else:
                            nc.vector.tensor_scalar(tu[:], ps[:], 1.0, None,
                                                    op0=mybir.AluOpType.add)
                        tmin = tmpp.tile([C, 512], F32, tag="tmin")
                        nc.scalar.activation(tmin[:], tu[:],
                                             mybir.ActivationFunctionType.Relu,
                                             scale=-1.0, bias=1.0)
                        texp = tmpp.tile([C, 512], F32, tag="texp")
                        nc.scalar.activation(texp[:], tmin[:],
                                             mybir.ActivationFunctionType.Exp,
                                             scale=-1.0)
                        for half in range(2):
                            pr = slice(half * D, (half + 1) * D)
                            nc.vector.tensor_max(dst[2 * m + half][:, cs],
                                                 texp[pr, 0:512], tu[pr, 0:512])

            # ---------------- P2: chunked causal linear attention ----------------
            # attnA: pairs 0,1 (feature-major (128, L)); attnB: pairs 2,3
            attnA = attnp.tile([C, L], BF16, tag="attnA")
            attnB = attnp.tile([C, L], BF16, tag="attnB")
            for p in range(4):
                att = attnA if p < 2 else attnB
                arow = slice((p % 2) * D, (p % 2) * D + D)
                vcols = slice(p * 65, p * 65 + 65)
                kcols = slice(p * D, (p + 1) * D)
                ou = p2p.tile([65, L], F32, tag=f"ou{p}", name=f"ou{p}", bufs=1)
                # per-chunk state terms T_c = K_c^T @ V^_c, all independent
                tsb = []
                for cI in range(NCH - 1):
                    tps = ps_sh.tile([D, 65], F32, tag="sh")
                    nc.tensor.matmul(tps[:], knat[cI][:, kcols],
                                     vnat[cI][:, vcols], start=True, stop=True)
                    ts = p2p.tile([D, 65], BF16, tag="ts", bufs=14,
                                  name=f"ts{p}_{cI}")
                    nc.scalar.copy(ts[:], tps[:])
                    tsb.append(ts)
                NB_CH = 4
                for cp2 in range(NCH // NB_CH):
                    cI0 = NB_CH * cp2
                    # A'[s,t] for NB_CH chunks side by side (causal mask s<=t)
                    aps = ps_ao.tile([C, NB_CH * C], F32, tag="ao")
                    for h in range(NB_CH):
                        cs = slice((cI0 + h) * C, (cI0 + h + 1) * C)
                        nc.tensor.matmul(aps[:, h * C:(h + 1) * C],
                                         kp_t[p][:, cs], qp_t[p][:, cs],
                                         start=True, stop=True)
                    asb = p2p.tile([C, NB_CH * C], BF16, tag="asb")
                    nc.vector.tensor_mul(asb[:], aps[:], mask2[:])
                    # O' (65, NB_CH*C) = V^.T @ A' + sum_{c'<c} T_c'.T @ Q'_c
                    ops = ps_ao.tile([65, NB_CH * C], F32, tag="ao")
                    for h in range(NB_CH):
                        cI = cI0 + h
                        cs = slice(cI * C, (cI + 1) * C)
                        hs = slice(h * C, (h + 1) * C)
                        nc.tensor.matmul(ops[:, hs], vnat[cI][:, vcols],
                                         asb[:, hs], start=True, stop=(cI == 0))
                        for c2 in range(cI):
                            nc.tensor.matmul(ops[:, hs], tsb[c2][:],
                                             qp_t[p][:, cs],
                                             start=False, stop=(c2 == cI - 1))
                    # stash unnormalized O' (incl. denominator row) for the pair
                    nc.scalar.copy(ou[:, cI0 * C:(cI0 + NB_CH) * C], ops[:])
                # batched division for the whole pair:
                # attn' = O'[0:64] * broadcast(1/(denom+eps))
                dn = p2p.tile([1, L], F32, tag="dn", bufs=2)
                nc.vector.tensor_scalar(dn[:], ou[64:65, :], EPS, None,
                                        op0=mybir.AluOpType.add)
                rcp = p2p.tile([1, L], F32, tag="rcp", bufs=2)
                nc.vector.reciprocal(rcp[:], dn[:])
                rbc = p2p.tile([D, L], F32, tag="rbc", bufs=2)
                for nbk in range(2):
                    cs2 = slice(nbk * 512, (nbk + 1) * 512)
                    rbc_ps = ps_ao.tile([D, 512], F32, tag="ao")
                    nc.tensor.matmul(rbc_ps[:], ones_row[:], rcp[:, cs2],
                                     start=True, stop=True)
                    nc.scalar.copy(rbc[:, cs2], rbc_ps[:])
                nc.vector.tensor_mul(att[arow, :], ou[0:64, :], rbc[:])

            # ---------------- A2A: redistribute attention outputs ----------------
            # shard j of the input buffer: (128 feats, l-chunk j), one DMA each
            a2a_outs = []
            for att in (attnA, attnB):
                a2a_in = dram.tile([N_CORES * C, C], BF16,
                                   tag=f"a2ain{len(a2a_outs)}")
                a2a_out = dram.tile([N_CORES * C, C], BF16,
                                    tag=f"a2aout{len(a2a_outs)}")
                nc.sync.dma_start(
                    out=a2a_in[:].rearrange("(j f) l -> f j l", j=N_CORES),
                    in_=att[:].rearrange("f (j l) -> f j l", j=N_CORES),
                )
                nc.gpsimd.collective_compute(
                    "AllToAll",
                    mybir.AluOpType.bypass,
                    replica_groups=[list(range(N_CORES))],
                    ins=[a2a_in.opt()],
                    outs=[a2a_out.opt()],
                )
                a2a_outs.append(a2a_out)

            # ---------------- woT load (single big cast-DMA) ----------------
            wo_all = big_load(woTp, woT, 8, E, "wo_all")
            wo_sb = [wo_all[:, k, :] for k in range(8)]

            # ---------------- P3: output projection ----------------
            for n in range(NB):
                # one DMA per half: 4 contiguous shards (cores n*4..n*4+3)
                ath = []
                for half in range(2):
                    t = atTp.tile([C, 4, C], BF16, tag=f"ath{n}_{half}",
                                  name=f"ath{n}_{half}")
                    nc.sync.dma_start(
                        out=t[:],
                        in_=a2a_outs[half][n * 4 * C:(n + 1) * 4 * C, :]
                            .rearrange("(s p) l -> p s l", p=C))
                    ath.append(t)
                at_tiles = [ath[kk % 2][:, kk // 2, :] for kk in range(8)]
                kk_order = [0, 2, 4, 6, 1, 3, 5, 7]
                osb = outp.tile([C, E], F32, tag=f"osb{n}")
                for nbk in range(2):
                    ps = ps_pj.tile([C, 512], F32, tag="pj")
                    for i, kk in enumerate(kk_order):
                        nc.tensor.matmul(ps[:], at_tiles[kk][:],
                                         wo_sb[kk][:, nbk * 512:(nbk + 1) * 512],
                                         start=(i == 0), stop=(i == 7))
                    cs = slice(nbk * 512, (nbk + 1) * 512)
                    if with_bias:
                        nc.vector.tensor_add(osb[:, cs], ps[:], bo_bc[:, cs])
                    else:
                        nc.scalar.copy(osb[:, cs], ps[:])
                    nc.sync.dma_start(out=out_d[n][:, cs], in_=osb[:, cs])
            nc.sync.dma_start(out=tok_o[:], in_=tok_i[:])

    _split_waits(nc)
    return nc


def _run_pjrt_timed(nc, in_maps, time_iters=0):
    """Replicates bass2jax.run_bass_via_pjrt's multi-core path, but keeps
    inputs device-resident and (optionally) times repeated executions.
    Returns (results, best_exec_seconds_or_None)."""
    import time as _time
    import jax
    from jax.sharding import Mesh, PartitionSpec, NamedSharding
    from jax.experimental.shard_map import shard_map
    from concourse import bass2jax, mybir as mb

    bass2jax.install_neuronx_cc_hook()
    n_cores = len(in_maps)
    partition_name = nc.partition_id_tensor.name if nc.partition_id_tensor else None

    in_names, out_names, out_avals, zero_outs = [], [], [], []
    for alloc in nc.m.functions[0].allocations:
        if not isinstance(alloc, mb.MemoryLocationSet):
            continue
        name = alloc.memorylocations[0].name
        if alloc.kind == "ExternalInput":
            if name != partition_name:
                in_names.append(name)
        elif alloc.kind == "ExternalOutput":
            out_names.append(name)
            shape = tuple(alloc.tensor_shape)
            dtype = mb.dt.np(alloc.dtype)
            out_avals.append(jax.core.ShapedArray(shape, dtype))
            zero_outs.append(np.zeros(shape, dtype))
    n_params = len(in_names)
    in_names.extend(out_names)
    if partition_name is not None:
        in_names.append(partition_name)

    chain = int(__import__("os").environ.get("TRN_KERNEL_CHAIN", "1"))
    tok_in_idx = in_names.index("tok") if "tok" in in_names else None
    tok_out_idx = out_names.index("tok_out") if "tok_out" in out_names else None

    def _body(*args):
        operands = list(args)
        pid = bass2jax.partition_id_tensor() if partition_name is not None else None
        outs = None
        for _ in range(chain):
            ops = list(operands)
            if outs is not None and tok_in_idx is not None:
                ops[tok_in_idx] = outs[tok_out_idx]  # serialize iterations
            if pid is not None:
                ops.append(pid)
            outs = bass2jax._bass_exec_p.bind(
                *ops,
                out_avals=tuple(out_avals),
                in_names=tuple(in_names),
                out_names=tuple(out_names),
                lowering_input_output_aliases=(),
                sim_require_finite=True,
                sim_require_nnan=True,
                nc=nc,
            )
        return tuple(outs)

    devices = jax.devices()[:n_cores]
    mesh = Mesh(np.asarray(devices), ("core",))
    in_specs = (PartitionSpec("core"),) * (n_params + len(out_names))
    out_specs = (PartitionSpec("core"),) * len(out_names)
    sharded = jax.jit(
        shard_map(_body, mesh=mesh, in_specs=in_specs, out_specs=out_specs,
                  check_rep=False),
        keep_unused=True,
    )
    per_core = [[np.asarray(m[name]) for name in in_names[:n_params]]
                for m in in_maps]
    concat_in = [np.concatenate([per_core[c][i] for c in range(n_cores)], axis=0)
                 for i in range(n_params)]
    concat_zeros = [np.zeros((n_cores * z.shape[0], *z.shape[1:]), z.dtype)
                    for z in zero_outs]
    shd = NamedSharding(mesh, PartitionSpec("core"))
    dev_in = [jax.device_put(a, shd) for a in concat_in + concat_zeros]

    out_arrs = sharded(*dev_in)
    jax.block_until_ready(out_arrs)
    best = None
    for _ in range(time_iters):
        t0 = _time.perf_counter()
        out_arrs2 = sharded(*dev_in)
        jax.block_until_ready(out_arrs2)
        dt = _time.perf_counter() - t0
        best = dt if best is None or dt < best else best
    results = [
        {name: np.asarray(out_arrs[i]).reshape(n_cores, *out_avals[i].shape)[c]
         for i, name in enumerate(out_names)}
        for c in range(n_cores)
    ]
    return results, best


def kernel(**inputs):
    global LAST_RESULT
    import os

    query = np.asarray(inputs["query"], np.float32)
    key = np.asarray(inputs["key"], np.float32)
    value = np.asarray(inputs["value"], np.float32)
    Wq = np.asarray(inputs["Wq"], np.float32)
    Wk = np.asarray(inputs["Wk"], np.float32)
    Wv = np.asarray(inputs["Wv"], np.float32)
    Wo = np.asarray(inputs["Wo"], np.float32)
    bq = np.asarray(inputs["bq"], np.float32)
    bk = np.asarray(inputs["bk"], np.float32)
    bv = np.asarray(inputs["bv"], np.float32)
    bo = np.asarray(inputs["bo"], np.float32)

    with_bias = any(np.any(b) for b in (bq, bk, bv, bo))
    nc = _build(with_bias)

    woT_full = np.ascontiguousarray(Wo.T)
    in_maps = []
    for c in range(N_CORES):
        a, b = c // 4, c % 4
        F = slice(FPC * b, FPC * (b + 1))
        m = {
            "xqT": np.ascontiguousarray(query[:, a, :].T),
            "xkT": np.ascontiguousarray(key[:, a, :].T),
            "xvT": np.ascontiguousarray(value[:, a, :].T),
            "wqT": np.ascontiguousarray(Wq[F, :].T),
            "wkT": np.ascontiguousarray(Wk[F, :].T),
            "wvT": np.ascontiguousarray(Wv[F, :].T),
            "woT": woT_full,
            "tok": np.zeros((1, 1), np.float32),
        }
        if with_bias:
            m["bq"] = np.ascontiguousarray(bq[F].reshape(FPC, 1))
            m["bk"] = np.ascontiguousarray(bk[F].reshape(FPC, 1))
            m["bv"] = np.ascontiguousarray(bv[F].reshape(FPC, 1))
            m["bo"] = np.ascontiguousarray(bo.reshape(1, E))
        in_maps.append(m)

    time_iters = int(os.environ.get("TRN_KERNEL_TIME_ITERS", "0"))
    results, best = _run_pjrt_timed(nc, in_maps, time_iters=time_iters)
    LAST_RESULT = {"results": results, "best_exec_s": best}

    out = np.empty((L, NB, E), np.float32)
    for c in range(N_CORES):
        o = results[c]["out"]  # (NB, C, E): my l-chunk rows for both batches
        for n in range(NB):
            out[c * C:(c + 1) * C, n, :] = o[n]
    return out
